# revision 85
# baseline (speedup 1.0000x reference)
"""Bass/Trainium2 kernel for nn_BakaAttention: 8-way data-parallel over batch.

Per core (one batch element):
  q = rope(x@wq, off=1024); k = rope(concat(past_k, x@wk), off=0); v = concat(past_v, x@wv)
  out = softmax(mask(q k^T / 16)) v @ wo

Layouts on chip: qT/kT are feature-major [f, t]; scores computed transposed
[s, t] so PV consumes probs directly as the moving operand. All matmuls run
in float32r (full PE rate at N>=256). New-v stays resident in SBUF (no DRAM
round trip); past_v streams per (TH, head). Softmax denominators accumulate
on the PE via a ones-column matmul per key block (no serial DVE chain).
Attention runs query-half (TH) outer / head inner so the o-projection for
TH=0 overlaps TH=1's attention; o-proj reads yT written in place over the
dead qT columns. Boundary (causal) key blocks compute only the live query
range; the diagonal 128-wide wedge is masked with a single tri multiply.
"""

import numpy as np

B, T, P, H, DH, DIN, DOUT = 8, 1024, 1024, 4, 256, 1024, 1152
S = P + T  # 2048 keys
THETA = 10000.0
NCORES = 8


def _host_constants():
    m = np.arange(0, DH, 2, dtype=np.float64) / DH          # 128 freqs
    inv = 1.0 / (THETA ** m)                                # [128]
    pos = np.arange(S, dtype=np.float64)                    # [2048]
    ang = np.outer(inv, pos)                                # [128, 2048]
    cos_full = np.cos(ang)
    sin_full = np.sin(ang)
    r = np.arange(128) // 2
    consts = {
        "cos_lo": cos_full[r, :].astype(np.float32),
        "cos_hi": cos_full[64 + r, :].astype(np.float32),
        "sin_lo": sin_full[r, :].astype(np.float32),
        "sin_hi": sin_full[64 + r, :].astype(np.float32),
    }
    prot = np.zeros((128, 128), np.float32)
    for mm in range(64):
        prot[2 * mm, 2 * mm + 1] = 1.0
        prot[2 * mm + 1, 2 * mm] = -1.0
    consts["prot"] = prot
    consts["ident"] = np.eye(128, dtype=np.float32)
    # masks2[:, 128:256] = tri (keep iff sl <= c); [:, 0:128] = zeros.
    # Boundary block ci<3 multiplies its diagonal 128 cols by tri;
    # ci==3 multiplies cols [256:512) by [zeros | tri].
    sl = np.arange(128)[:, None]
    c = np.arange(128)[None, :]
    tri = (sl <= c).astype(np.float32)
    masks2 = np.concatenate([np.zeros((128, 128), np.float32), tri], axis=1)
    consts["masks2"] = np.ascontiguousarray(masks2)
    consts["ones"] = np.ones((128, 4), np.float32)
    consts["onesr"] = np.ones((1, 128), np.float32)
    return consts


def build_kernel(debug=False):
    import concourse.bass as bass
    import concourse.mybir as mybir
    from concourse import bacc
    from concourse.tile import TileContext

    f32 = mybir.dt.float32
    f32r = mybir.dt.float32r
    AF = mybir.ActivationFunctionType
    OP = mybir.AluOpType

    nc = bacc.Bacc(None, target_bir_lowering=False)

    x_d = nc.dram_tensor("x", [T, DIN], f32r, kind="ExternalInput")
    pk_d = nc.dram_tensor("past_k", [P, H, DH], f32r, kind="ExternalInput")
    pv_d = nc.dram_tensor("past_v", [P, H, DH], f32r, kind="ExternalInput")
    wq_d = nc.dram_tensor("wq", [DIN, DIN], f32r, kind="ExternalInput")
    wk_d = nc.dram_tensor("wk", [DIN, DIN], f32r, kind="ExternalInput")
    wv_d = nc.dram_tensor("wv", [DIN, DIN], f32r, kind="ExternalInput")
    wo_d = nc.dram_tensor("wo", [DIN, DOUT], f32r, kind="ExternalInput")
    cos_lo_d = nc.dram_tensor("cos_lo", [128, S], f32, kind="ExternalInput")
    cos_hi_d = nc.dram_tensor("cos_hi", [128, S], f32, kind="ExternalInput")
    sin_lo_d = nc.dram_tensor("sin_lo", [128, S], f32, kind="ExternalInput")
    sin_hi_d = nc.dram_tensor("sin_hi", [128, S], f32, kind="ExternalInput")
    prot_d = nc.dram_tensor("prot", [128, 128], f32r, kind="ExternalInput")
    ident_d = nc.dram_tensor("ident", [128, 128], f32r, kind="ExternalInput")
    masks2_d = nc.dram_tensor("masks2", [128, 256], f32, kind="ExternalInput")
    ones_d = nc.dram_tensor("ones", [128, 4], f32r, kind="ExternalInput")
    onesr_d = nc.dram_tensor("onesr", [1, 128], f32r, kind="ExternalInput")
    out_d = nc.dram_tensor("out", [T, DOUT], f32, kind="ExternalOutput")
    vkind = dict(kind="ExternalOutput") if debug else {}
    v_r = nc.dram_tensor("v_r", [T, DIN], f32r, **vkind)
    if debug:
        qT_dump = nc.dram_tensor("qT_dump", [8, 128, T], f32r, kind="ExternalOutput")
        kT_dump = nc.dram_tensor("kT_dump", [8, 128, S], f32r, kind="ExternalOutput")

    from contextlib import ExitStack
    stack = ExitStack()
    with TileContext(nc) as tc, stack:
        cstp = stack.enter_context(tc.tile_pool(name="consts", bufs=1))
        prot = cstp.tile([128, 128], f32r, name="prot", tag="prot")
        ident = cstp.tile([128, 128], f32r, name="ident", tag="ident")
        ones_sb = cstp.tile([128, 4], f32r, name="ones_sb", tag="ones_sb")
        onesr_sb = cstp.tile([1, 128], f32r, name="onesr_sb", tag="onesr_sb")
        # ident gates the first transpose — first on the sync queue; the
        # rest ride gpsimd so x loads start immediately after.
        nc.sync.dma_start(out=ident[:], in_=ident_d[:])
        nc.gpsimd.dma_start(out=prot[:], in_=prot_d[:])
        nc.gpsimd.dma_start(out=ones_sb[:], in_=ones_d[:])
        nc.gpsimd.dma_start(out=onesr_sb[:], in_=onesr_d[:])

        resid = stack.enter_context(tc.tile_pool(name="resid", bufs=1))
        kT = [resid.tile([128, S], f32r, name=f"kT{i}", tag=f"kT{i}") for i in range(8)]
        qTp = stack.enter_context(tc.tile_pool(name="qTp", bufs=1))
        qT = [qTp.tile([128, T], f32r, name=f"qT{i}", tag=f"qT{i}") for i in range(8)]

        # ---------------- Phase A-E: xT, projections, rope ----------------
        # First-head v tiles live at the BOTTOM of the right-side heap,
        # below every projection-phase pool: their DMAs have no WAR on
        # projection SBUF and can land during phase C, so the first PV
        # matmul of the attention never waits.
        pvnv0p = stack.enter_context(
            tc.tile_pool(name="pvnv0", bufs=1, side="right"))
        pvt0 = [pvnv0p.tile([128, DH], f32r, name=f"pvt0_{j}", tag=f"pvt0_{j}")
                for j in range(8)]
        nv0 = [pvnv0p.tile([128, DH], f32r, name=f"nv0_{j}", tag=f"nv0_{j}")
               for j in range(4)]

        with tc.tile_pool(name="tables", bufs=1) as tabp, \
             tc.tile_pool(name="p2", bufs=2) as p2p:
            cos_t = [tabp.tile([128, T], f32, name="clo", tag="clo"),
                     tabp.tile([128, T], f32, name="chi", tag="chi")]
            sin_t = [tabp.tile([128, T], f32, name="slo", tag="slo"),
                     tabp.tile([128, T], f32, name="shi", tag="shi")]

            def load_tables(p0, eng=None):
                # Table RELOADS have a WAR on the previous phase's rope reads;
                # issue those from the scalar queue so they can't head-of-line
                # block the streaming sync queue. Parity-0 (lo) tables first:
                # the ft0/ft1 ropes gate the first attention scores.
                eng = eng or nc.sync
                eng.dma_start(out=cos_t[0][:], in_=cos_lo_d[:, p0:p0 + T])
                eng.dma_start(out=sin_t[0][:], in_=sin_lo_d[:, p0:p0 + T])
                eng.dma_start(out=cos_t[1][:], in_=cos_hi_d[:, p0:p0 + T])
                eng.dma_start(out=sin_t[1][:], in_=sin_hi_d[:, p0:p0 + T])

            def rope_combine(dst_ap, raw_sb, rot_ps, ft, off, n):
                # dst = raw * cos + rot * sin ; table rows by f-tile parity
                ctab = cos_t[ft % 2][:, off:off + n]
                stab = sin_t[ft % 2][:, off:off + n]
                t1 = p2p.tile([128, 512], f32, name="ropet1", tag="ropet1")
                nc.gpsimd.tensor_tensor(t1[:, :n], raw_sb, ctab, op=OP.mult)
                t2 = p2p.tile([128, 512], f32, name="ropet2", tag="ropet2")
                nc.vector.tensor_tensor(t2[:, :n], rot_ps, stab, op=OP.mult)
                nc.vector.tensor_tensor(dst_ap, t1[:, :n], t2[:, :n], op=OP.add)

            with tc.tile_pool(name="p2rot", bufs=2, space="PSUM") as rotps:
                with tc.tile_pool(name="p2xT", bufs=1) as xtp:
                    xT = [xtp.tile([128, T], f32r, name=f"xT{i}", tag=f"xT{i}")
                          for i in range(8)]
                    # -- A: transpose x into xT (PE transpose, 1.5 cyc/row).
                    # second half is emitted from inside the first k-proj
                    # group, which only needs query columns 0:512.
                    with tc.tile_pool(name="xldp", bufs=2, side="right") as xlp, \
                         tc.tile_pool(name="p2st", bufs=3, side="right") as stp, \
                         tc.tile_pool(name="p2raw", bufs=1, side="right") as rawp, \
                         tc.tile_pool(name="p2tpa", bufs=2, space="PSUM") as tppa, \
                         tc.tile_pool(name="p2ps", bufs=4, space="PSUM") as ps2:
                        def emit_A(tts):
                            for tt in tts:
                                xt = xlp.tile([128, DIN], f32r, name="xload", tag="xload")
                                if tt < 2:
                                    # split so the first transposes start after
                                    # a quarter tile instead of the whole tile
                                    for q in range(2):
                                        nc.sync.dma_start(
                                            out=xt[:, 512 * q:512 * (q + 1)],
                                            in_=x_d[128 * tt:128 * (tt + 1),
                                                    512 * q:512 * (q + 1)])
                                else:
                                    # tail tiles ride gpsimd: only wg0 is
                                    # ahead of them there, so they land long
                                    # before the th=1 projection needs them
                                    eng = nc.sync if tt < 4 else nc.gpsimd
                                    eng.dma_start(out=xt[:], in_=x_d[128 * tt:128 * (tt + 1), :])
                                for kt in range(8):
                                    tp = tppa.tile([128, 128], f32r, name="tps", tag="tps")
                                    nc.tensor.transpose(tp[:], xt[:, 128 * kt:128 * (kt + 1)], ident[:])
                                    nc.scalar.copy(xT[kt][:, 128 * tt:128 * (tt + 1)], tp[:])
                                if tt == 3:
                                    load_tables(P)  # positions 1024..2047 for q, new-k
                        emit_A(range(0, 4))

                        # -- B,E: k then q projections (transposed layout) +
                        # rope. kt-accumulation per f-tile (half-group of 2
                        # PSUM banks); each f-tile's copy+rotate+combine
                        # epilogue is emitted one half-group later so the PE
                        # never waits on the PSUM-draining copies.
                        pending_rot = None
                        a_tail = [True]

                        def make_rot(dst, ft, psl2):
                            def emit():
                                raw = rawp.tile([128, 1024], f32r, name="rawsb", tag="rawsb")
                                for th in range(2):
                                    nc.scalar.copy(raw[:, 512 * th:512 * (th + 1)],
                                                   psl2[th][:])
                                for th in range(2):
                                    rp = rotps.tile([128, 512], f32, name="rotps", tag="rotps")
                                    nc.tensor.matmul(rp[:], prot[:].bitcast(f32r),
                                                     raw[:, 512 * th:512 * (th + 1)].bitcast(f32r),
                                                     start=True, stop=True)
                                    off = P if dst is kT else 0
                                    dst_ap = dst[ft][:, off + 512 * th:off + 512 * (th + 1)]
                                    rope_combine(dst_ap, raw[:, 512 * th:512 * (th + 1)],
                                                 rp[:], ft, 512 * th, 512)
                            return emit

                        # past_k rows are transposed straight into
                        # kT[:, 0:1024] from inside the projection loop.
                        # The transposes are NOT bursted: each is queued and
                        # popped between projection matmuls so its LDWEIGHTS
                        # hides under a full 512-col matmul instead of the
                        # tiny 128-col transpose in front of it.
                        from collections import deque
                        pend_tp = deque()

                        def emit_pk(st):
                            pkt = stp.tile([128, H * DH], f32r, name="pkload",
                                           tag="pkload", bufs=2)
                            nc.sync.dma_start(out=pkt[:],
                                              in_=pk_d[128 * st:128 * (st + 1), :, :])

                            def one(ft):
                                def emit():
                                    tp = tppa.tile([128, 128], f32r, name="tps",
                                                   tag="tps")
                                    nc.tensor.transpose(
                                        tp[:], pkt[:, 128 * ft:128 * (ft + 1)],
                                        ident[:])
                                    nc.scalar.copy(
                                        kT[ft][:, 128 * st:128 * (st + 1)], tp[:])
                                return emit
                            for ft in range(8):
                                pend_tp.append(one(ft))

                        # wq/wk stream in 256-col groups, prefetched one full
                        # group ahead through a 16-slot ring so the PE never
                        # waits on a weight load.
                        glist = [(wk_d, g) for g in range(4)] + \
                                [(wq_d, g) for g in range(4)]

                        def load_wgroup(i):
                            gw_d, g = glist[i]
                            # group 0 rides the idle gpsimd queue at startup so
                            # it lands in parallel with the 4MB x load on sync
                            eng = nc.gpsimd if i == 0 else nc.sync
                            tiles = []
                            for kt in range(8):
                                wt = xlp.tile([128, 256], f32r, name="wload",
                                              tag="wload", bufs=16)
                                eng.dma_start(
                                    out=wt[:],
                                    in_=gw_d[128 * kt:128 * (kt + 1),
                                             256 * g:256 * (g + 1)])
                                tiles.append(wt)
                            return tiles

                        wts_cur = load_wgroup(0)
                        gidx = 0
                        gseq = 0
                        for w_d, dst in ((wk_d, kT), (wq_d, qT)):
                            for ftg in range(4):
                                wts = wts_cur
                                gseq += 1
                                if gseq < 8:
                                    wts_cur = load_wgroup(gseq)
                                for f2 in range(2):
                                    ft = 2 * ftg + f2
                                    psl2 = [ps2.tile([128, 512], f32, name=f"pj{2 * f2 + th}",
                                                     tag=f"pj{2 * f2 + th}", bufs=1)
                                            for th in range(2)]
                                    for th in range(2):
                                        if th == 1 and a_tail[0]:
                                            emit_A(range(4, 8))
                                            a_tail[0] = False
                                        for kt in range(8):
                                            nc.tensor.matmul(
                                                psl2[th][:],
                                                wts[kt][:, 128 * f2:128 * (f2 + 1)].bitcast(f32r),
                                                xT[kt][:, 512 * th:512 * (th + 1)].bitcast(f32r),
                                                start=(kt == 0), stop=(kt == 7))
                                            if th == 0 and kt == 2 and pending_rot is not None:
                                                pending_rot()
                                                pending_rot = None
                                            elif kt % 2 == 1 and pend_tp:
                                                pend_tp.popleft()()
                                    if gidx % 2 == 0:
                                        emit_pk(gidx // 2)
                                    gidx += 1
                                    pending_rot = make_rot(dst, ft, psl2)
                        pending_rot()
                        pending_rot = None
                        while pend_tp:
                            pend_tp.popleft()()

                    # -- D1: past_k transposed straight into kT[:, 0:1024];
                    # then v-projection interleaved with the past-k rope so
                    # PE (v matmuls), DVE+GpSimd (rope combines) all stay busy.
                    with tc.tile_pool(name="wvres", bufs=2, side="right") as wvr:
                        wvh = [[None] * 8, [None] * 8]

                        def load_wvh(fh):
                            for kt in range(8):
                                w = wvr.tile([128, 512], f32r, name=f"wvh{kt}",
                                             tag=f"wvh{kt}")
                                nc.sync.dma_start(
                                    out=w[:],
                                    in_=wv_d[128 * kt:128 * (kt + 1),
                                             512 * fh:512 * (fh + 1)])
                                wvh[fh][kt] = w

                        load_wvh(0)
                        load_tables(0, eng=nc.scalar)  # positions 0..1023
                        for j in range(8):
                            nc.sync.dma_start(out=pvt0[j][:],
                                              in_=pv_d[128 * j:128 * (j + 1), 0, :])
                        load_wvh(1)

                        rope_left = [(ft, sh) for ft in range(8) for sh in range(2)]

                        def emit_ropes(n):
                            for _ in range(n):
                                if not rope_left:
                                    return
                                ft, sh = rope_left.pop(0)
                                rp = rotps.tile([128, 512], f32, name="rotps", tag="rotps")
                                nc.tensor.matmul(rp[:], prot[:].bitcast(f32r),
                                                 kT[ft][:, 512 * sh:512 * (sh + 1)],
                                                 start=True, stop=True)
                                rope_combine(kT[ft][:, 512 * sh:512 * (sh + 1)],
                                             kT[ft][:, 512 * sh:512 * (sh + 1)],
                                             rp[:], ft, 512 * sh, 512)

                        # -- C: v projection [s, f] -> DRAM, + rope interleave --
                        with tc.tile_pool(name="pvps", bufs=2, space="PSUM") as pvp:
                            rope_sched = [0, 2, 3, 3, 3, 2, 2, 1]
                            for fh in range(2):
                                for stg in range(4):
                                    psl = [pvp.tile([128, 512], f32, name=f"pv{s2}",
                                                    tag=f"pv{s2}")
                                           for s2 in range(2)]
                                    for kt in range(8):
                                        for s2 in range(2):
                                            st = 2 * stg + s2
                                            nc.tensor.matmul(
                                                psl[s2][:],
                                                xT[kt][:, 128 * st:128 * (st + 1)].bitcast(f32r),
                                                wvh[fh][kt][:],
                                                start=(kt == 0), stop=(kt == 7))
                                    emit_ropes(rope_sched[4 * fh + stg])
                                    for s2 in range(2):
                                        st = 2 * stg + s2
                                        vsb = p2p.tile([128, 512], f32r, name="vsb",
                                                       tag="vsb", bufs=2)
                                        nc.scalar.copy(vsb[:], psl[s2][:])
                                        nc.sync.dma_start(
                                            out=v_r[128 * st:128 * (st + 1),
                                                    512 * fh:512 * (fh + 1)],
                                            in_=vsb[:])
                                    if fh == 1 and stg == 1:
                                        # first-head new-v tiles (rows 0..511
                                        # complete in both halves by now)
                                        for j in range(4):
                                            nc.sync.dma_start(
                                                out=nv0[j][:],
                                                in_=v_r[128 * j:128 * (j + 1), 0:DH])
                            emit_ropes(16)

        if debug:
            for i in range(8):
                nc.sync.dma_start(out=kT_dump[i], in_=kT[i][:])
                nc.sync.dma_start(out=qT_dump[i], in_=qT[i][:])

        # ---------------- Phase F: attention + fused o-projection ----------
        # TH (query half) outer, head inner. yT is written in place over the
        # dead qT columns; o-proj for TH=0 is emitted interleaved into TH=1's
        # j-streams so the PE never drains.
        mskp = stack.enter_context(tc.tile_pool(name="p3msk", bufs=1))
        masks2 = mskp.tile([128, 256], f32, name="masks2", tag="masks2")
        nc.sync.dma_start(out=masks2[:], in_=masks2_d[:])
        wop = stack.enter_context(tc.tile_pool(name="p4wo", bufs=1))
        wo_sb = [wop.tile([128, DOUT], f32r, name=f"wo{i}", tag=f"wo{i}")
                 for i in range(8)]

        with tc.tile_pool(name="p3sm", bufs=2) as smp, \
             tc.tile_pool(name="p3sc", bufs=3, space="PSUM") as scps, \
             tc.tile_pool(name="p3smps", bufs=1, space="PSUM") as smps_p, \
             tc.tile_pool(name="p3y", bufs=2, space="PSUM") as yps, \
             tc.tile_pool(name="p3nv", bufs=2) as nvp, \
             tc.tile_pool(name="p3pv", bufs=2) as pvtp, \
             tc.tile_pool(name="probs", bufs=4) as prp, \
             tc.tile_pool(name="p4o", bufs=2) as osp:

            def load_nv(h, cnt):
                # new-v tiles for head h: j=8..8+cnt → v_r rows, head-h cols
                nv = [nvp.tile([128, DH], f32r, name=f"nv{j}", tag=f"nv{j}")
                      for j in range(cnt)]
                for j in range(cnt):
                    nc.sync.dma_start(
                        out=nv[j][:],
                        in_=v_r[128 * j:128 * (j + 1), DH * h:DH * (h + 1)])
                return nv

            def load_pv(h):
                # past_v tiles for head h: j=0..7
                pvt = [pvtp.tile([128, DH], f32r, name=f"pvt{j}", tag=f"pvt{j}")
                       for j in range(8)]
                for j in range(8):
                    nc.sync.dma_start(out=pvt[j][:],
                                      in_=pv_d[128 * j:128 * (j + 1), h, :])
                return pvt

            pending_epi = None
            oproj_tasks = []  # list of closures, each one (tt, ds) chunk
            out_tiles = {}

            def make_epi(h, TH, rcr, ytp_ps):
                # deferred epilogue: broadcast the reciprocal denominators
                # and normalize into yT (written over the dead qT columns).
                # The PSUM-freeing recip/rcr part runs inline after the j-loop.
                def epi():
                    bc = scps.tile([128, 512], f32, name="bc", tag="sc")
                    nc.tensor.matmul(bc[:], onesr_sb[:], rcr[:],
                                     start=True, stop=True)
                    bc_sb = smp.tile([128, 512], f32, name="bcsb", tag="bcsb")
                    nc.scalar.copy(bc_sb[:], bc[:])
                    for fb in range(2):
                        nc.vector.tensor_tensor(
                            qT[2 * h + fb][:, 512 * TH:512 * (TH + 1)],
                            ytp_ps[fb][:],
                            bc_sb[:],
                            op=OP.mult)
                return epi

            def make_oproj(tt, ds):
                def task():
                    if tt not in out_tiles:
                        out_tiles[tt] = osp.tile([128, DOUT], f32, name="osb",
                                                 tag="osb")
                    ot = out_tiles[tt]
                    op_ps = scps.tile([128, 384], f32, name="ops", tag="sc")
                    for fk in range(8):
                        nc.tensor.matmul(
                            op_ps[:],
                            qT[fk][:, 128 * tt:128 * (tt + 1)],
                            wo_sb[fk][:, 384 * ds:384 * (ds + 1)],
                            start=(fk == 0), stop=(fk == 7))
                    nc.scalar.copy(ot[:, 384 * ds:384 * (ds + 1)], op_ps[:])
                    if ds == 2:
                        nc.sync.dma_start(out=out_d[128 * tt:128 * (tt + 1), :],
                                          in_=ot[:])
                        del out_tiles[tt]
                return task

            pvt = pvt0
            nv = nv0
            # wo destinations are WAR-blocked until the projection pools
            # free; issue from the gpsimd queue (idle in attention) so they
            # don't head-of-line block the pv/nv prefetch stream.
            for kt in range(8):
                nc.gpsimd.dma_start(out=wo_sb[kt][:], in_=wo_d[128 * kt:128 * (kt + 1), :])
            nv_next = [None]
            pv_next = [None]
            blk = 0  # global (TH, h) block index 0..7
            for TH in range(2):
                for h in range(4):
                    jmax = 12 + 4 * TH
                    ytp_ps = [yps.tile([128, 512], f32, name=f"ytp{i}",
                                       tag=f"ytp{i}") for i in range(2)]
                    sm = smps_p.tile([1, 512], f32, name="smps", tag="smps")

                    def lohi(j):
                        ci = j - (8 + 4 * TH)
                        lo = 0 if ci < 0 else (128 * ci if ci < 3 else 256)
                        return ci, lo

                    sc_tiles = {}
                    qbuf = []
                    quad_a = [None]
                    pending_ones = [None]

                    def emit_sc(j):
                        ci, lo = lohi(j)
                        sc = scps.tile([128, 512], f32, name="sc", tag="sc")
                        for fk in range(2):
                            nc.tensor.matmul(
                                sc[:, lo:],
                                kT[2 * h + fk][:, 128 * j:128 * (j + 1)].bitcast(f32r),
                                qT[2 * h + fk][:, 512 * TH + lo:512 * (TH + 1)].bitcast(f32r),
                                start=(fk == 0), stop=(fk == 1))
                        sc_tiles[j] = sc

                    # scores run two key-blocks ahead of exp/PV so the exp
                    # latency never blocks the PE's instruction queue.
                    emit_sc(0)
                    emit_sc(1)
                    for j in range(jmax):
                        ci, lo = lohi(j)
                        sc = sc_tiles.pop(j)
                        pj = prp.tile([128, 512], f32r, name="pj", tag="pj")
                        nc.scalar.activation(pj[:, lo:], sc[:, lo:], AF.Exp,
                                             scale=float(DH ** -0.5))
                        if ci >= 0:
                            if ci < 3:
                                nc.vector.tensor_tensor(
                                    pj[:, 128 * ci:128 * (ci + 1)],
                                    pj[:, 128 * ci:128 * (ci + 1)],
                                    masks2[:, 128:256], op=OP.mult)
                            else:
                                nc.vector.tensor_tensor(
                                    pj[:, 256:512], pj[:, 256:512],
                                    masks2[:, 0:256], op=OP.mult)
                        if j + 2 < jmax:
                            emit_sc(j + 2)
                        # deferred quad-denominator matmul from the previous
                        # iteration: the PE meets it well after the DVE tree
                        if pending_ones[0] is not None:
                            pending_ones[0]()
                            pending_ones[0] = None
                        # softmax denominator on the PE: ones-column matmul.
                        # Full (unmasked) blocks are quad-summed on the idle
                        # vector engine first (pair + pair + tree-top), so one
                        # PE matmul covers four key blocks.
                        if ci < 0:
                            qbuf.append(pj)
                            if len(qbuf) == 2:
                                pa = prp.tile([128, 512], f32r, name="pjs",
                                              tag="pjs", bufs=2)
                                nc.vector.tensor_tensor(pa[:], qbuf[0][:],
                                                        qbuf[1][:], op=OP.add)
                                quad_a[0] = pa
                            elif len(qbuf) == 4:
                                pb = prp.tile([128, 512], f32r, name="pjs",
                                              tag="pjs", bufs=2)
                                nc.vector.tensor_tensor(pb[:], qbuf[2][:],
                                                        qbuf[3][:], op=OP.add)
                                pa = quad_a[0]
                                nc.vector.tensor_tensor(pa[:], pa[:], pb[:],
                                                        op=OP.add)
                                first = (j == 3)

                                def ones_quad(pa=pa, first=first):
                                    nc.tensor.matmul(sm[:], ones_sb[:, 0:1],
                                                     pa[:], start=first,
                                                     stop=False,
                                                     skip_group_check=True)
                                pending_ones[0] = ones_quad
                                qbuf.clear()
                                quad_a[0] = None
                        else:
                            nc.tensor.matmul(sm[:, lo:], ones_sb[:, 0:1], pj[:, lo:],
                                             start=False, stop=(j == jmax - 1),
                                             skip_group_check=True)
                        for fb in range(2):
                            if j < 8:
                                va = pvt[j][:, 128 * fb:128 * (fb + 1)]
                            else:
                                va = nv[j - 8][:, 128 * fb:128 * (fb + 1)]
                            nc.tensor.matmul(
                                ytp_ps[fb][:, lo:],
                                va,
                                pj[:, lo:],
                                start=(j == 0), stop=(j == jmax - 1),
                                skip_group_check=True)
                        # software-pipelined emissions
                        if j == 2 and pending_epi is not None:
                            pending_epi()
                            pending_epi = None
                        if j == 5 and nv_next[0] is None and blk < 7:
                            nTH = TH if h < 3 else TH + 1
                            pv_next[0] = load_pv((h + 1) % 4)
                            nv_next[0] = load_nv((h + 1) % 4, 4 + 4 * nTH)
                        if j in (4, 7, 10, 13) and oproj_tasks:
                            oproj_tasks.pop(0)()
                    # inline epilogue head: reciprocal of the denominators
                    # (frees the sm PSUM bank before the next head starts)
                    rc = smp.tile([1, 512], f32, name="rc", tag="rc", bufs=1)
                    nc.vector.reciprocal_approx_fast(rc[:], sm[:])
                    rcr = smp.tile([1, 512], f32r, name="rcr", tag="rcr", bufs=1)
                    nc.scalar.copy(rcr[:], rc[:])
                    pending_epi = make_epi(h, TH, rcr, ytp_ps)
                    if nv_next[0] is not None:
                        nv = nv_next[0]
                        nv_next[0] = None
                    if pv_next[0] is not None:
                        pvt = pv_next[0]
                        pv_next[0] = None
                    blk += 1
                for tt in range(4 * TH, 4 * TH + 4):
                    for ds in range(3):
                        oproj_tasks.append(make_oproj(tt, ds))

            # drain: last epilogue + TH=1's o-projection
            pending_epi()
            pending_epi = None
            for t in oproj_tasks:
                t()
            oproj_tasks = []

    nc.finalize()
    return nc


_NC_CACHE = {}


def run(x, past_k, past_v, wq, wk, wv, wo, debug=False, trace=False):
    from concourse.bass_utils import run_bass_kernel_spmd

    key = (debug,)
    if key not in _NC_CACHE:
        _NC_CACHE[key] = build_kernel(debug=debug)
    nc = _NC_CACHE[key]
    consts = _host_constants()
    in_maps = []
    for b in range(NCORES):
        m = {
            "x": np.ascontiguousarray(x[b]),
            "past_k": np.ascontiguousarray(past_k[b]),
            "past_v": np.ascontiguousarray(past_v[b]),
            "wq": wq, "wk": wk, "wv": wv, "wo": wo,
            "cos_lo": consts["cos_lo"], "cos_hi": consts["cos_hi"],
            "sin_lo": consts["sin_lo"], "sin_hi": consts["sin_hi"],
            "prot": consts["prot"], "ident": consts["ident"],
            "masks2": consts["masks2"], "ones": consts["ones"],
            "onesr": consts["onesr"],
        }
        in_maps.append(m)
    res = run_bass_kernel_spmd(nc, in_maps, list(range(NCORES)), trace=trace)
    out = np.stack([res.results[b]["out"] for b in range(NCORES)], axis=0)
    return out, res


def kernel(x, past_k, past_v, wq, wk, wv, wo):
    out, _ = run(x, past_k, past_v, wq, wk, wv, wo)
    return out


# revision 91
# speedup vs baseline: 1.0036x; 1.0036x over previous
"""Bass/Trainium2 kernel for nn_BakaAttention: 8-way data-parallel over batch.

Per core (one batch element):
  q = rope(x@wq, off=1024); k = rope(concat(past_k, x@wk), off=0); v = concat(past_v, x@wv)
  out = softmax(mask(q k^T / 16)) v @ wo

Layouts on chip: qT/kT are feature-major [f, t]; scores computed transposed
[s, t] so PV consumes probs directly as the moving operand. All matmuls run
in float32r (full PE rate at N>=256). New-v stays resident in SBUF (no DRAM
round trip); past_v streams per (TH, head). Softmax denominators accumulate
on the PE via a ones-column matmul per key block (no serial DVE chain).
Attention runs query-half (TH) outer / head inner so the o-projection for
TH=0 overlaps TH=1's attention; o-proj reads yT written in place over the
dead qT columns. Boundary (causal) key blocks compute only the live query
range; the diagonal 128-wide wedge is masked with a single tri multiply.
"""

import numpy as np

B, T, P, H, DH, DIN, DOUT = 8, 1024, 1024, 4, 256, 1024, 1152
S = P + T  # 2048 keys
THETA = 10000.0
NCORES = 8


def _host_constants():
    m = np.arange(0, DH, 2, dtype=np.float64) / DH          # 128 freqs
    inv = 1.0 / (THETA ** m)                                # [128]
    pos = np.arange(S, dtype=np.float64)                    # [2048]
    ang = np.outer(inv, pos)                                # [128, 2048]
    cos_full = np.cos(ang)
    sin_full = np.sin(ang)
    r = np.arange(128) // 2
    consts = {
        "cos_lo": cos_full[r, :].astype(np.float32),
        "cos_hi": cos_full[64 + r, :].astype(np.float32),
        "sin_lo": sin_full[r, :].astype(np.float32),
        "sin_hi": sin_full[64 + r, :].astype(np.float32),
    }
    prot = np.zeros((128, 128), np.float32)
    for mm in range(64):
        prot[2 * mm, 2 * mm + 1] = 1.0
        prot[2 * mm + 1, 2 * mm] = -1.0
    consts["prot"] = prot
    consts["ident"] = np.eye(128, dtype=np.float32)
    # masks2[:, 128:256] = tri (keep iff sl <= c); [:, 0:128] = zeros.
    # Boundary block ci<3 multiplies its diagonal 128 cols by tri;
    # ci==3 multiplies cols [256:512) by [zeros | tri].
    sl = np.arange(128)[:, None]
    c = np.arange(128)[None, :]
    tri = (sl <= c).astype(np.float32)
    masks2 = np.concatenate([np.zeros((128, 128), np.float32), tri], axis=1)
    consts["masks2"] = np.ascontiguousarray(masks2)
    consts["ones"] = np.ones((128, 4), np.float32)
    consts["onesr"] = np.ones((1, 128), np.float32)
    return consts


def build_kernel(debug=False):
    import concourse.bass as bass
    import concourse.mybir as mybir
    from concourse import bacc
    from concourse.tile import TileContext

    f32 = mybir.dt.float32
    f32r = mybir.dt.float32r
    AF = mybir.ActivationFunctionType
    OP = mybir.AluOpType

    nc = bacc.Bacc(None, target_bir_lowering=False)

    x_d = nc.dram_tensor("x", [T, DIN], f32r, kind="ExternalInput")
    pk_d = nc.dram_tensor("past_k", [P, H, DH], f32r, kind="ExternalInput")
    pv_d = nc.dram_tensor("past_v", [P, H, DH], f32r, kind="ExternalInput")
    wq_d = nc.dram_tensor("wq", [DIN, DIN], f32r, kind="ExternalInput")
    wk_d = nc.dram_tensor("wk", [DIN, DIN], f32r, kind="ExternalInput")
    wv_d = nc.dram_tensor("wv", [DIN, DIN], f32r, kind="ExternalInput")
    wo_d = nc.dram_tensor("wo", [DIN, DOUT], f32r, kind="ExternalInput")
    cos_lo_d = nc.dram_tensor("cos_lo", [128, S], f32, kind="ExternalInput")
    cos_hi_d = nc.dram_tensor("cos_hi", [128, S], f32, kind="ExternalInput")
    sin_lo_d = nc.dram_tensor("sin_lo", [128, S], f32, kind="ExternalInput")
    sin_hi_d = nc.dram_tensor("sin_hi", [128, S], f32, kind="ExternalInput")
    prot_d = nc.dram_tensor("prot", [128, 128], f32r, kind="ExternalInput")
    ident_d = nc.dram_tensor("ident", [128, 128], f32r, kind="ExternalInput")
    masks2_d = nc.dram_tensor("masks2", [128, 256], f32, kind="ExternalInput")
    ones_d = nc.dram_tensor("ones", [128, 4], f32r, kind="ExternalInput")
    onesr_d = nc.dram_tensor("onesr", [1, 128], f32r, kind="ExternalInput")
    out_d = nc.dram_tensor("out", [T, DOUT], f32, kind="ExternalOutput")
    vkind = dict(kind="ExternalOutput") if debug else {}
    v_r = nc.dram_tensor("v_r", [T, DIN], f32r, **vkind)
    if debug:
        qT_dump = nc.dram_tensor("qT_dump", [8, 128, T], f32r, kind="ExternalOutput")
        kT_dump = nc.dram_tensor("kT_dump", [8, 128, S], f32r, kind="ExternalOutput")

    from contextlib import ExitStack
    stack = ExitStack()
    with TileContext(nc) as tc, stack:
        cstp = stack.enter_context(tc.tile_pool(name="consts", bufs=1))
        prot = cstp.tile([128, 128], f32r, name="prot", tag="prot")
        ident = cstp.tile([128, 128], f32r, name="ident", tag="ident")
        ones_sb = cstp.tile([128, 4], f32r, name="ones_sb", tag="ones_sb")
        onesr_sb = cstp.tile([1, 128], f32r, name="onesr_sb", tag="onesr_sb")
        # ident gates the first transpose — first on the sync queue; the
        # rest ride gpsimd so x loads start immediately after.
        nc.sync.dma_start(out=ident[:], in_=ident_d[:])
        nc.gpsimd.dma_start(out=prot[:], in_=prot_d[:])
        nc.gpsimd.dma_start(out=ones_sb[:], in_=ones_d[:])
        nc.gpsimd.dma_start(out=onesr_sb[:], in_=onesr_d[:])

        resid = stack.enter_context(tc.tile_pool(name="resid", bufs=1))
        kT = [resid.tile([128, S], f32r, name=f"kT{i}", tag=f"kT{i}") for i in range(8)]
        qTp = stack.enter_context(tc.tile_pool(name="qTp", bufs=1))
        qT = [qTp.tile([128, T], f32r, name=f"qT{i}", tag=f"qT{i}") for i in range(8)]

        # ---------------- Phase A-E: xT, projections, rope ----------------
        # First-head v tiles live at the BOTTOM of the right-side heap,
        # below every projection-phase pool: their DMAs have no WAR on
        # projection SBUF and can land during phase C, so the first PV
        # matmul of the attention never waits.
        pvnv0p = stack.enter_context(
            tc.tile_pool(name="pvnv0", bufs=1, side="right"))
        pvt0 = pvnv0p.tile([128, 8 * DH], f32r, name="pvt0", tag="pvt0")
        nv0 = pvnv0p.tile([128, 4 * DH], f32r, name="nv0", tag="nv0")

        with tc.tile_pool(name="tables", bufs=1) as tabp, \
             tc.tile_pool(name="p2", bufs=2) as p2p:
            cos_t = [tabp.tile([128, T], f32, name="clo", tag="clo"),
                     tabp.tile([128, T], f32, name="chi", tag="chi")]
            sin_t = [tabp.tile([128, T], f32, name="slo", tag="slo"),
                     tabp.tile([128, T], f32, name="shi", tag="shi")]

            def load_tables(p0, eng=None):
                # Table RELOADS have a WAR on the previous phase's rope reads;
                # issue those from the scalar queue so they can't head-of-line
                # block the streaming sync queue. Parity-0 (lo) tables first:
                # the ft0/ft1 ropes gate the first attention scores.
                eng = eng or nc.sync
                eng.dma_start(out=cos_t[0][:], in_=cos_lo_d[:, p0:p0 + T])
                eng.dma_start(out=sin_t[0][:], in_=sin_lo_d[:, p0:p0 + T])
                eng.dma_start(out=cos_t[1][:], in_=cos_hi_d[:, p0:p0 + T])
                eng.dma_start(out=sin_t[1][:], in_=sin_hi_d[:, p0:p0 + T])

            def rope_combine(dst_ap, raw_sb, rot_ps, ft, off, n):
                # dst = raw * cos + rot * sin ; table rows by f-tile parity
                ctab = cos_t[ft % 2][:, off:off + n]
                stab = sin_t[ft % 2][:, off:off + n]
                t1 = p2p.tile([128, 512], f32, name="ropet1", tag="ropet1")
                nc.gpsimd.tensor_tensor(t1[:, :n], raw_sb, ctab, op=OP.mult)
                t2 = p2p.tile([128, 512], f32, name="ropet2", tag="ropet2")
                nc.vector.tensor_tensor(t2[:, :n], rot_ps, stab, op=OP.mult)
                nc.vector.tensor_tensor(dst_ap, t1[:, :n], t2[:, :n], op=OP.add)

            with tc.tile_pool(name="p2rot", bufs=2, space="PSUM") as rotps:
                with tc.tile_pool(name="p2xT", bufs=1) as xtp:
                    xT = [xtp.tile([128, T], f32r, name=f"xT{i}", tag=f"xT{i}")
                          for i in range(8)]
                    # -- A: transpose x into xT (PE transpose, 1.5 cyc/row).
                    # second half is emitted from inside the first k-proj
                    # group, which only needs query columns 0:512.
                    with tc.tile_pool(name="xldp", bufs=2, side="right") as xlp, \
                         tc.tile_pool(name="p2st", bufs=3, side="right") as stp, \
                         tc.tile_pool(name="p2raw", bufs=1, side="right") as rawp, \
                         tc.tile_pool(name="p2tpa", bufs=2, space="PSUM") as tppa, \
                         tc.tile_pool(name="p2ps", bufs=4, space="PSUM") as ps2:
                        def emit_A(tts):
                            for tt in tts:
                                xt = xlp.tile([128, DIN], f32r, name="xload", tag="xload")
                                if tt < 2:
                                    # split so the first transposes start after
                                    # a quarter tile instead of the whole tile
                                    for q in range(2):
                                        nc.sync.dma_start(
                                            out=xt[:, 512 * q:512 * (q + 1)],
                                            in_=x_d[128 * tt:128 * (tt + 1),
                                                    512 * q:512 * (q + 1)])
                                else:
                                    # tail tiles ride gpsimd: only wg0 is
                                    # ahead of them there, so they land long
                                    # before the th=1 projection needs them
                                    eng = nc.sync if tt < 4 else nc.gpsimd
                                    eng.dma_start(out=xt[:], in_=x_d[128 * tt:128 * (tt + 1), :])
                                for kt in range(8):
                                    tp = tppa.tile([128, 128], f32r, name="tps", tag="tps")
                                    nc.tensor.transpose(tp[:], xt[:, 128 * kt:128 * (kt + 1)], ident[:])
                                    nc.scalar.copy(xT[kt][:, 128 * tt:128 * (tt + 1)], tp[:])
                                if tt == 3:
                                    load_tables(P)  # positions 1024..2047 for q, new-k
                        emit_A(range(0, 4))

                        # -- B,E: k then q projections (transposed layout) +
                        # rope. kt-accumulation per f-tile (half-group of 2
                        # PSUM banks); each f-tile's copy+rotate+combine
                        # epilogue is emitted one half-group later so the PE
                        # never waits on the PSUM-draining copies.
                        pending_rot = None
                        a_tail = [True]

                        def make_rot(dst, ft, psl2):
                            def emit():
                                raw = rawp.tile([128, 1024], f32r, name="rawsb", tag="rawsb")
                                for th in range(2):
                                    nc.scalar.copy(raw[:, 512 * th:512 * (th + 1)],
                                                   psl2[th][:])
                                for th in range(2):
                                    rp = rotps.tile([128, 512], f32, name="rotps", tag="rotps")
                                    nc.tensor.matmul(rp[:], prot[:].bitcast(f32r),
                                                     raw[:, 512 * th:512 * (th + 1)].bitcast(f32r),
                                                     start=True, stop=True)
                                    off = P if dst is kT else 0
                                    dst_ap = dst[ft][:, off + 512 * th:off + 512 * (th + 1)]
                                    rope_combine(dst_ap, raw[:, 512 * th:512 * (th + 1)],
                                                 rp[:], ft, 512 * th, 512)
                            return emit

                        # past_k rows are transposed straight into
                        # kT[:, 0:1024] from inside the projection loop.
                        # The transposes are NOT bursted: each is queued and
                        # popped between projection matmuls so its LDWEIGHTS
                        # hides under a full 512-col matmul instead of the
                        # tiny 128-col transpose in front of it.
                        from collections import deque
                        pend_tp = deque()

                        def emit_pk(st):
                            pkt = stp.tile([128, H * DH], f32r, name="pkload",
                                           tag="pkload", bufs=2)
                            nc.sync.dma_start(out=pkt[:],
                                              in_=pk_d[128 * st:128 * (st + 1), :, :])

                            def one(ft):
                                def emit():
                                    tp = tppa.tile([128, 128], f32r, name="tps",
                                                   tag="tps")
                                    nc.tensor.transpose(
                                        tp[:], pkt[:, 128 * ft:128 * (ft + 1)],
                                        ident[:])
                                    nc.scalar.copy(
                                        kT[ft][:, 128 * st:128 * (st + 1)], tp[:])
                                return emit
                            for ft in range(8):
                                pend_tp.append(one(ft))

                        # wq/wk stream in 256-col groups, prefetched one full
                        # group ahead through a 16-slot ring so the PE never
                        # waits on a weight load.
                        glist = [(wk_d, g) for g in range(4)] + \
                                [(wq_d, g) for g in range(4)]

                        def load_wgroup(i):
                            gw_d, g = glist[i]
                            # group 0 rides the idle gpsimd queue at startup so
                            # it lands in parallel with the 4MB x load on sync
                            eng = nc.gpsimd if i == 0 else nc.sync
                            tiles = []
                            for kt in range(8):
                                wt = xlp.tile([128, 256], f32r, name="wload",
                                              tag="wload", bufs=16)
                                eng.dma_start(
                                    out=wt[:],
                                    in_=gw_d[128 * kt:128 * (kt + 1),
                                             256 * g:256 * (g + 1)])
                                tiles.append(wt)
                            return tiles

                        wts_cur = load_wgroup(0)
                        gidx = 0
                        gseq = 0
                        for w_d, dst in ((wk_d, kT), (wq_d, qT)):
                            for ftg in range(4):
                                wts = wts_cur
                                gseq += 1
                                if gseq < 8:
                                    wts_cur = load_wgroup(gseq)
                                for f2 in range(2):
                                    ft = 2 * ftg + f2
                                    psl2 = [ps2.tile([128, 512], f32, name=f"pj{2 * f2 + th}",
                                                     tag=f"pj{2 * f2 + th}", bufs=1)
                                            for th in range(2)]
                                    for th in range(2):
                                        if th == 1 and a_tail[0]:
                                            emit_A(range(4, 8))
                                            a_tail[0] = False
                                        for kt in range(8):
                                            nc.tensor.matmul(
                                                psl2[th][:],
                                                wts[kt][:, 128 * f2:128 * (f2 + 1)].bitcast(f32r),
                                                xT[kt][:, 512 * th:512 * (th + 1)].bitcast(f32r),
                                                start=(kt == 0), stop=(kt == 7))
                                            if th == 0 and kt == 2 and pending_rot is not None:
                                                pending_rot()
                                                pending_rot = None
                                            elif kt % 2 == 1 and pend_tp:
                                                pend_tp.popleft()()
                                    if gidx % 2 == 0:
                                        emit_pk(gidx // 2)
                                    gidx += 1
                                    pending_rot = make_rot(dst, ft, psl2)
                        pending_rot()
                        pending_rot = None
                        while pend_tp:
                            pend_tp.popleft()()

                    # -- D1: past_k transposed straight into kT[:, 0:1024];
                    # then v-projection interleaved with the past-k rope so
                    # PE (v matmuls), DVE+GpSimd (rope combines) all stay busy.
                    with tc.tile_pool(name="wvres", bufs=2, side="right") as wvr:
                        wvh = [[None] * 8, [None] * 8]

                        def load_wvh(fh):
                            # 4 strided DMAs of 2 kt-tiles each: few issues
                            # (0.65us of engine time apiece) but still fine-
                            # grained enough that the first matmuls don't
                            # wait on the whole 2MB
                            w = wvr.tile([128, 8 * 512], f32r, name="wvh",
                                         tag="wvh")
                            for q in range(4):
                                nc.sync.dma_start(
                                    out=w[:, 1024 * q:1024 * (q + 1)]
                                        .rearrange("p (k d) -> p k d", k=2),
                                    in_=wv_d[256 * q:256 * (q + 1),
                                             512 * fh:512 * (fh + 1)].rearrange(
                                        "(k p) d -> p k d", p=128))
                            for kt in range(8):
                                wvh[fh][kt] = w[:, 512 * kt:512 * (kt + 1)]

                        load_wvh(0)
                        load_tables(0, eng=nc.scalar)  # positions 0..1023
                        nc.sync.dma_start(
                            out=pvt0[:].rearrange("p (j d) -> p j d", j=8),
                            in_=pv_d[:, 0, :].rearrange("(j p) d -> p j d", p=128))
                        load_wvh(1)

                        rope_left = [(ft, sh) for ft in range(8) for sh in range(2)]

                        def emit_ropes(n):
                            for _ in range(n):
                                if not rope_left:
                                    return
                                ft, sh = rope_left.pop(0)
                                rp = rotps.tile([128, 512], f32, name="rotps", tag="rotps")
                                nc.tensor.matmul(rp[:], prot[:].bitcast(f32r),
                                                 kT[ft][:, 512 * sh:512 * (sh + 1)],
                                                 start=True, stop=True)
                                rope_combine(kT[ft][:, 512 * sh:512 * (sh + 1)],
                                             kT[ft][:, 512 * sh:512 * (sh + 1)],
                                             rp[:], ft, 512 * sh, 512)

                        # -- C: v projection [s, f] -> DRAM, + rope interleave --
                        with tc.tile_pool(name="pvps", bufs=2, space="PSUM") as pvp:
                            rope_sched = [0, 2, 3, 3, 3, 2, 2, 1]
                            for fh in range(2):
                                for stg in range(4):
                                    psl = [pvp.tile([128, 512], f32, name=f"pv{s2}",
                                                    tag=f"pv{s2}")
                                           for s2 in range(2)]
                                    for kt in range(8):
                                        for s2 in range(2):
                                            st = 2 * stg + s2
                                            nc.tensor.matmul(
                                                psl[s2][:],
                                                xT[kt][:, 128 * st:128 * (st + 1)].bitcast(f32r),
                                                wvh[fh][kt][:],
                                                start=(kt == 0), stop=(kt == 7))
                                    emit_ropes(rope_sched[4 * fh + stg])
                                    for s2 in range(2):
                                        st = 2 * stg + s2
                                        vsb = p2p.tile([128, 512], f32r, name="vsb",
                                                       tag="vsb", bufs=2)
                                        nc.scalar.copy(vsb[:], psl[s2][:])
                                        nc.sync.dma_start(
                                            out=v_r[128 * st:128 * (st + 1),
                                                    512 * fh:512 * (fh + 1)],
                                            in_=vsb[:])
                                    if fh == 1 and stg == 1:
                                        # first-head new-v tiles (rows 0..511
                                        # complete in both halves by now)
                                        nc.sync.dma_start(
                                            out=nv0[:].rearrange(
                                                "p (j d) -> p j d", j=4),
                                            in_=v_r[0:512, 0:DH].rearrange(
                                                "(j p) d -> p j d", p=128))
                            emit_ropes(16)

        if debug:
            for i in range(8):
                nc.sync.dma_start(out=kT_dump[i], in_=kT[i][:])
                nc.sync.dma_start(out=qT_dump[i], in_=qT[i][:])

        # ---------------- Phase F: attention + fused o-projection ----------
        # TH (query half) outer, head inner. yT is written in place over the
        # dead qT columns; o-proj for TH=0 is emitted interleaved into TH=1's
        # j-streams so the PE never drains.
        mskp = stack.enter_context(tc.tile_pool(name="p3msk", bufs=1))
        masks2 = mskp.tile([128, 256], f32, name="masks2", tag="masks2")
        nc.sync.dma_start(out=masks2[:], in_=masks2_d[:])
        wop = stack.enter_context(tc.tile_pool(name="p4wo", bufs=1))
        wo_sb = [wop.tile([128, DOUT], f32r, name=f"wo{i}", tag=f"wo{i}")
                 for i in range(8)]

        with tc.tile_pool(name="p3sm", bufs=2) as smp, \
             tc.tile_pool(name="p3sc", bufs=3, space="PSUM") as scps, \
             tc.tile_pool(name="p3smps", bufs=1, space="PSUM") as smps_p, \
             tc.tile_pool(name="p3y", bufs=2, space="PSUM") as yps, \
             tc.tile_pool(name="p3nv", bufs=2) as nvp, \
             tc.tile_pool(name="p3pv", bufs=2) as pvtp, \
             tc.tile_pool(name="probs", bufs=4) as prp, \
             tc.tile_pool(name="p4o", bufs=2) as osp:

            def load_nv(h, cnt):
                # new-v for head h: j=8..8+cnt → v_r rows, head-h cols.
                # single strided DMA (the ~0.65us issue cost dominates)
                nv = nvp.tile([128, 8 * DH], f32r, name="nv", tag="nv")
                nc.sync.dma_start(
                    out=nv[:, :cnt * DH].rearrange("p (j d) -> p j d", j=cnt),
                    in_=v_r[0:128 * cnt, DH * h:DH * (h + 1)].rearrange(
                        "(j p) d -> p j d", p=128))
                return nv

            def load_pv(h):
                # past_v for head h: j=0..7, one strided DMA
                pvt = pvtp.tile([128, 8 * DH], f32r, name="pvt", tag="pvt")
                nc.sync.dma_start(
                    out=pvt[:].rearrange("p (j d) -> p j d", j=8),
                    in_=pv_d[:, h, :].rearrange("(j p) d -> p j d", p=128))
                return pvt

            pending_epi = None
            oproj_tasks = []  # list of closures, each one (tt, ds) chunk
            out_tiles = {}

            def make_epi(h, TH, rcr, ytp_ps):
                # deferred epilogue: broadcast the reciprocal denominators
                # and normalize into yT (written over the dead qT columns).
                # The PSUM-freeing recip/rcr part runs inline after the j-loop.
                def epi():
                    bc = scps.tile([128, 512], f32, name="bc", tag="sc")
                    nc.tensor.matmul(bc[:], onesr_sb[:], rcr[:],
                                     start=True, stop=True)
                    bc_sb = smp.tile([128, 512], f32, name="bcsb", tag="bcsb")
                    nc.scalar.copy(bc_sb[:], bc[:])
                    for fb in range(2):
                        nc.vector.tensor_tensor(
                            qT[2 * h + fb][:, 512 * TH:512 * (TH + 1)],
                            ytp_ps[fb][:],
                            bc_sb[:],
                            op=OP.mult)
                return epi

            def make_oproj(tt, ds):
                def task():
                    if tt not in out_tiles:
                        out_tiles[tt] = osp.tile([128, DOUT], f32, name="osb",
                                                 tag="osb")
                    ot = out_tiles[tt]
                    op_ps = scps.tile([128, 384], f32, name="ops", tag="sc")
                    for fk in range(8):
                        nc.tensor.matmul(
                            op_ps[:],
                            qT[fk][:, 128 * tt:128 * (tt + 1)],
                            wo_sb[fk][:, 384 * ds:384 * (ds + 1)],
                            start=(fk == 0), stop=(fk == 7))
                    nc.scalar.copy(ot[:, 384 * ds:384 * (ds + 1)], op_ps[:])
                    if ds == 2:
                        nc.sync.dma_start(out=out_d[128 * tt:128 * (tt + 1), :],
                                          in_=ot[:])
                        del out_tiles[tt]
                return task

            pvt = pvt0
            nv = nv0
            # wo destinations are WAR-blocked until the projection pools
            # free; issue from the gpsimd queue (idle in attention) so they
            # don't head-of-line block the pv/nv prefetch stream.
            for kt in range(8):
                nc.gpsimd.dma_start(out=wo_sb[kt][:], in_=wo_d[128 * kt:128 * (kt + 1), :])
            nv_next = [None]
            pv_next = [None]
            blk = 0  # global (TH, h) block index 0..7
            for TH in range(2):
                for h in range(4):
                    jmax = 12 + 4 * TH
                    ytp_ps = [yps.tile([128, 512], f32, name=f"ytp{i}",
                                       tag=f"ytp{i}") for i in range(2)]
                    sm = smps_p.tile([1, 512], f32, name="smps", tag="smps")

                    def lohi(j):
                        ci = j - (8 + 4 * TH)
                        lo = 0 if ci < 0 else (128 * ci if ci < 3 else 256)
                        return ci, lo

                    sc_tiles = {}
                    qbuf = []
                    quad_a = [None]
                    pending_ones = [None]

                    def emit_sc(j):
                        ci, lo = lohi(j)
                        sc = scps.tile([128, 512], f32, name="sc", tag="sc")
                        for fk in range(2):
                            nc.tensor.matmul(
                                sc[:, lo:],
                                kT[2 * h + fk][:, 128 * j:128 * (j + 1)].bitcast(f32r),
                                qT[2 * h + fk][:, 512 * TH + lo:512 * (TH + 1)].bitcast(f32r),
                                start=(fk == 0), stop=(fk == 1))
                        sc_tiles[j] = sc

                    # scores run two key-blocks ahead of exp/PV so the exp
                    # latency never blocks the PE's instruction queue.
                    emit_sc(0)
                    emit_sc(1)
                    for j in range(jmax):
                        ci, lo = lohi(j)
                        sc = sc_tiles.pop(j)
                        pj = prp.tile([128, 512], f32r, name="pj", tag="pj")
                        nc.scalar.activation(pj[:, lo:], sc[:, lo:], AF.Exp,
                                             scale=float(DH ** -0.5))
                        if ci >= 0:
                            if ci < 3:
                                nc.vector.tensor_tensor(
                                    pj[:, 128 * ci:128 * (ci + 1)],
                                    pj[:, 128 * ci:128 * (ci + 1)],
                                    masks2[:, 128:256], op=OP.mult)
                            else:
                                nc.vector.tensor_tensor(
                                    pj[:, 256:512], pj[:, 256:512],
                                    masks2[:, 0:256], op=OP.mult)
                        if j + 2 < jmax:
                            emit_sc(j + 2)
                        # deferred quad-denominator matmul from the previous
                        # iteration: the PE meets it well after the DVE tree
                        if pending_ones[0] is not None:
                            pending_ones[0]()
                            pending_ones[0] = None
                        # softmax denominator on the PE: ones-column matmul.
                        # Full (unmasked) blocks are quad-summed on the idle
                        # vector engine first (pair + pair + tree-top), so one
                        # PE matmul covers four key blocks.
                        if ci < 0:
                            qbuf.append(pj)
                            if len(qbuf) == 2:
                                pa = prp.tile([128, 512], f32r, name="pjs",
                                              tag="pjs", bufs=2)
                                nc.vector.tensor_tensor(pa[:], qbuf[0][:],
                                                        qbuf[1][:], op=OP.add)
                                quad_a[0] = pa
                            elif len(qbuf) == 4:
                                pb = prp.tile([128, 512], f32r, name="pjs",
                                              tag="pjs", bufs=2)
                                nc.vector.tensor_tensor(pb[:], qbuf[2][:],
                                                        qbuf[3][:], op=OP.add)
                                pa = quad_a[0]
                                nc.vector.tensor_tensor(pa[:], pa[:], pb[:],
                                                        op=OP.add)
                                first = (j == 3)

                                def ones_quad(pa=pa, first=first):
                                    nc.tensor.matmul(sm[:], ones_sb[:, 0:1],
                                                     pa[:], start=first,
                                                     stop=False,
                                                     skip_group_check=True)
                                pending_ones[0] = ones_quad
                                qbuf.clear()
                                quad_a[0] = None
                        else:
                            nc.tensor.matmul(sm[:, lo:], ones_sb[:, 0:1], pj[:, lo:],
                                             start=False, stop=(j == jmax - 1),
                                             skip_group_check=True)
                        for fb in range(2):
                            if j < 8:
                                va = pvt[:, 256 * j + 128 * fb:
                                         256 * j + 128 * (fb + 1)]
                            else:
                                va = nv[:, 256 * (j - 8) + 128 * fb:
                                        256 * (j - 8) + 128 * (fb + 1)]
                            nc.tensor.matmul(
                                ytp_ps[fb][:, lo:],
                                va,
                                pj[:, lo:],
                                start=(j == 0), stop=(j == jmax - 1),
                                skip_group_check=True)
                        # software-pipelined emissions
                        if j == 2 and pending_epi is not None:
                            pending_epi()
                            pending_epi = None
                        if j == 5 and nv_next[0] is None and blk < 7:
                            nTH = TH if h < 3 else TH + 1
                            pv_next[0] = load_pv((h + 1) % 4)
                            nv_next[0] = load_nv((h + 1) % 4, 4 + 4 * nTH)
                        if j in (4, 7, 10, 13) and oproj_tasks:
                            oproj_tasks.pop(0)()
                    # inline epilogue head: reciprocal of the denominators
                    # (frees the sm PSUM bank before the next head starts)
                    rc = smp.tile([1, 512], f32, name="rc", tag="rc", bufs=1)
                    nc.vector.reciprocal_approx_fast(rc[:], sm[:])
                    rcr = smp.tile([1, 512], f32r, name="rcr", tag="rcr", bufs=1)
                    nc.scalar.copy(rcr[:], rc[:])
                    pending_epi = make_epi(h, TH, rcr, ytp_ps)
                    if nv_next[0] is not None:
                        nv = nv_next[0]
                        nv_next[0] = None
                    if pv_next[0] is not None:
                        pvt = pv_next[0]
                        pv_next[0] = None
                    blk += 1
                for tt in range(4 * TH, 4 * TH + 4):
                    for ds in range(3):
                        oproj_tasks.append(make_oproj(tt, ds))

            # drain: last epilogue + TH=1's o-projection
            pending_epi()
            pending_epi = None
            for t in oproj_tasks:
                t()
            oproj_tasks = []

    nc.finalize()
    return nc


_NC_CACHE = {}


def run(x, past_k, past_v, wq, wk, wv, wo, debug=False, trace=False):
    from concourse.bass_utils import run_bass_kernel_spmd

    key = (debug,)
    if key not in _NC_CACHE:
        _NC_CACHE[key] = build_kernel(debug=debug)
    nc = _NC_CACHE[key]
    consts = _host_constants()
    in_maps = []
    for b in range(NCORES):
        m = {
            "x": np.ascontiguousarray(x[b]),
            "past_k": np.ascontiguousarray(past_k[b]),
            "past_v": np.ascontiguousarray(past_v[b]),
            "wq": wq, "wk": wk, "wv": wv, "wo": wo,
            "cos_lo": consts["cos_lo"], "cos_hi": consts["cos_hi"],
            "sin_lo": consts["sin_lo"], "sin_hi": consts["sin_hi"],
            "prot": consts["prot"], "ident": consts["ident"],
            "masks2": consts["masks2"], "ones": consts["ones"],
            "onesr": consts["onesr"],
        }
        in_maps.append(m)
    res = run_bass_kernel_spmd(nc, in_maps, list(range(NCORES)), trace=trace)
    out = np.stack([res.results[b]["out"] for b in range(NCORES)], axis=0)
    return out, res


def kernel(x, past_k, past_v, wq, wk, wv, wo):
    out, _ = run(x, past_k, past_v, wq, wk, wv, wo)
    return out
